# revision 30
# baseline (speedup 1.0000x reference)
"""Trainium2 Bass kernel for DynamicSpatialAttention.

reference semantics (per sample b):
  pooled = x.mean((2,3))                       [C]
  z      = relu(pooled @ w1 + b1)              [C]
  kern   = (z @ w2 + b2).reshape(3,3)          per-sample 3x3 kernel
  m      = x.mean(1)                           [H,W]   channel-mean map
  att    = sigmoid(conv2d(m, kern, pad=1))     [H,W]
  out    = x * att[None]

Distribution: data-parallel over batch B across 8 NeuronCores (4 samples
per core, fully independent -- no collectives).

The problem is pure memory streaming; the 2e-2 tolerance leaves
precision headroom, so BOTH directions ride bf16: the host casts x to
bf16 before upload (identical numerics to the previous on-DMA cast --
the device always computed in bf16) and out is stored bf16 (host
upcasts).  HBM traffic: 33.5 MB read + 33.5 MB write per core => ~190us
roofline at ~350 GB/s.

Schedule: loads stream on the gpsimd SWDGE queue in 1 MiB chunks;
stores ride the two HWDGE rings (sync/scalar), overlapping the next
samples' loads.  With bf16 loads HBM is far from saturated, so the
read/write mixing penalty is small, and store timing has ~70us of slack
before it blocks the 5-deep xres buffer ring (WAR).

Hard-won scheduler lessons encoded here:
- chain_b ops are high-priority AND carry an explicit order-only dep on
  mults_{b-1}: the static per-engine scheduler otherwise weaves chain_b
  (whose DMA-dependent inputs it mispredicts as early) into the
  multiply stream of b-1, head-of-line blocking the store path.
- the m->[h,w] reshape is done with 4 one-hot selector matmuls on
  TensorE (~1.5us) instead of a 128-descriptor SBUF->SBUF DMA (15.5us
  measured).
- engine balance per ~55us sample window: DVE ~47us (2 pooled partials,
  PSUM->SBUF staging of broadcast s, 2x all-bf16 multiplies), ACT ~36us
  (6 pooled partials via accum_out side-sums, 4 stagings, chain
  activations), PE ~48us (one-hot channel-sum matmuls, K=1 broadcast
  matmuls, selector reshape, chain matmuls).
"""

import numpy as np

B, C, H, W = 32, 256, 128, 128
HW = H * W
KS = 3
N_CORES = 8
BS = B // N_CORES


def build_nc(bs=BS, c=C, h=H, w=W):
    import concourse.bass as bass  # noqa: F401
    import concourse.tile as tile
    from concourse.tile_rust import add_dep_helper
    from concourse import bacc, mybir
    from concourse.masks import make_identity

    f32 = mybir.dt.float32
    bf16 = mybir.dt.bfloat16
    AX = mybir.AxisListType
    AF = mybir.ActivationFunctionType

    hw = h * w
    assert c == 256, "kernel assumes 2 channel halves of 128"
    QW = 512                      # msum free dim / broadcast-matmul width
    NQ = hw // QW                 # number of 512-wide hw chunks (rows of msum)
    assert NQ <= 32
    CH = 4096                     # load chunk free width (1 MiB bf16 per DMA)
    NCH = hw // CH                # load chunks per sample-half
    QPC = CH // QW                # 512-chunks per load chunk
    MW = 1024                     # multiply/bps granularity
    NM = hw // MW                 # multiply segments per half
    SW = 4096                     # store chunk free width (1 MiB per DMA)
    NS = hw // SW                 # store chunks per half
    SRW = hw // 2                 # staged-s cols per partition (rows 0 and 64)
    QPR = SRW // QW               # 512-chunks per staged-s partition
    RPB = h // (hw // QW)         # h-rows per msum row (128/32 = 4)

    nc = bacc.Bacc("TRN2", target_bir_lowering=False, debug=False)
    x_d = nc.declare_dram_parameter("x", [bs, c, hw], bf16, isOutput=False)
    w1_d = nc.declare_dram_parameter("w1", [c, c], f32, isOutput=False)
    b1_d = nc.declare_dram_parameter("b1", [c], f32, isOutput=False)
    w2_d = nc.declare_dram_parameter("w2", [c, KS * KS], f32, isOutput=False)
    b2_d = nc.declare_dram_parameter("b2", [KS * KS], f32, isOutput=False)
    out_d = nc.declare_dram_parameter("out", [bs, c, hw], bf16, isOutput=True)

    with tile.TileContext(nc) as tc:
        with (
            tc.tile_pool(name="xr", bufs=21) as xr,
            tc.tile_pool(name="scs", bufs=1) as scs,
            tc.tile_pool(name="srp", bufs=1) as srp,
            tc.tile_pool(name="bsp", bufs=2) as bsp,
            tc.tile_pool(name="small", bufs=2) as small,
            tc.tile_pool(name="singles", bufs=1) as singles,
            tc.tile_pool(name="convt", bufs=2) as convt,
            tc.tile_pool(name="pm", bufs=3, space="PSUM") as pm,
            tc.tile_pool(name="pb", bufs=2, space="PSUM") as pb,
            tc.tile_pool(name="ps", bufs=1, space="PSUM") as ps,
        ):
            # ---- constants / weights (loaded once) ----
            estrip = singles.tile([128, 2 * NQ], bf16)
            nc.vector.memset(estrip, 0.0)
            nc.vector.memset(estrip[:, NQ : NQ + 1], 1.0)
            ones_r = singles.tile([128, 128], bf16)
            nc.vector.memset(ones_r, 1.0)
            ones_rf = singles.tile([1, 128], f32)
            nc.vector.memset(ones_rf, 1.0)
            # 0/1 diagonal masks used to build the banded conv matrices:
            # ident[h,h']=d(h'=h), d_up[h,:]=e_{h+1}, d_dn[h,:]=e_{h-1}
            ident = singles.tile([h, h], bf16)
            make_identity(nc, ident)
            d_up = singles.tile([h, h], bf16)
            d_dn = singles.tile([h, h], bf16)
            nc.vector.memset(d_up, 0.0)
            nc.vector.memset(d_dn, 0.0)
            # sync ring, NOT gpsimd: these wait on make_identity's memsets,
            # and at the head of the gpsimd queue they would delay every
            # sample-0 load descriptor behind them
            nc.sync.dma_start(out=d_up[0 : h - 1, :], in_=ident[1:h, :])
            nc.sync.dma_start(out=d_dn[1:h, :], in_=ident[0 : h - 1, :])
            # selector masks for the msum [NQ,512] -> m [h,w] reshape on
            # TensorE: sel[r][q, i] = 1 iff i == RPB*q + r, so
            # sel[r].T @ m32[:, 128r:128(r+1)] scatters m32 row q to
            # partition RPB*q + r
            sels = []
            for r in range(RPB):
                sel = singles.tile([NQ, h], bf16, tag=f"sel{r}")
                nc.gpsimd.memset(sel, 0.0)
                nc.gpsimd.affine_select(
                    out=sel,
                    in_=sel,
                    compare_op=mybir.AluOpType.not_equal,
                    fill=1.0,
                    base=-r,
                    # iota = -r - RPB*q + i; == 0 exactly at i = RPB*q + r
                    pattern=[[1, h]],
                    channel_multiplier=-RPB,
                )
                sels.append(sel)
            w1_sb = singles.tile([128, 2, c], f32)  # [i_part, i_blk, j]
            nc.sync.dma_start(
                out=w1_sb, in_=w1_d.rearrange("(ib i) j -> i ib j", ib=2)
            )
            # fold the 1/HW of the spatial mean into w1 so pooled can stay
            # a raw sum (one chain hop less per sample)
            nc.scalar.activation(
                out=w1_sb, in_=w1_sb, func=AF.Copy, scale=1.0 / hw
            )
            w2_sb = singles.tile([128, 2, KS * KS], f32)  # [j_part, j_blk, t]
            nc.sync.dma_start(
                out=w2_sb, in_=w2_d.rearrange("(jb j) t -> j jb t", jb=2)
            )
            b1_sb = singles.tile([128, 2], f32)
            nc.sync.dma_start(
                out=b1_sb, in_=b1_d.rearrange("(jb j) -> j jb", jb=2)
            )
            b2_sb = singles.tile([1, KS * KS], f32)
            nc.sync.dma_start(
                out=b2_sb, in_=b2_d.rearrange("(o t) -> o t", o=1)
            )

            def emit_loads(b):
                """Loads (gpsimd SWDGE, bf16) + chansum matmuls + pooled
                partial accumulation (all ScalarE) as chunks land.
                xres is a ring of chunk-granular tiles so a load only
                WAR-waits on the store of the chunk 21 positions back."""
                msum = pm.tile([NQ, QW], f32, tag="msum", name="msum")
                parts = small.tile([128, 2 * NCH + 3], f32, tag="parts")
                xres = {}
                i_mm = 0
                n_mm = 2 * NCH * QPC
                for hh in range(2):
                    for q in range(NCH):
                        t = xr.tile([128, CH], bf16, tag="xres", name="xres")
                        xres[hh, q] = t
                        # the sample's LAST chunk loads as two 2048 halves:
                        # its pooled partial gates the chain, and the split
                        # pipelines the partial against the landing (~2us
                        # off each production boundary)
                        lastc = hh == 1 and q == NCH - 1
                        for ph in range(4 if lastc else 1):
                            pw = CH // 4 if lastc else CH
                            nc.gpsimd.dma_start(
                                out=t[:, pw * ph : pw * (ph + 1)],
                                in_=x_d[
                                    b,
                                    128 * hh : 128 * (hh + 1),
                                    CH * q + pw * ph : CH * q + pw * (ph + 1),
                                ],
                            )
                        # pooled partial sums per 4096-chunk: mostly on
                        # ScalarE (accum_out side-sums of dummy copies) to
                        # keep VectorE free for staging + multiplies
                        # ALL partials on ScalarE: a DVE-resident partial
                        # whose chunk is ring-stalled head-of-line blocks
                        # the multiply stream (11.7us gap measured)
                        # NOTE: ordering these partials after the previous
                        # chain's sigmoid (any variant) measured WORSE
                        # (+25-40us): productions are serially coupled, so
                        # delaying partial starts cascades.  Leave the
                        # scheduler free here.
                        pi = NCH * hh + q
                        for ph in range(4 if lastc else 1):
                            pw = CH // 4 if lastc else CH
                            sc = scs.tile([128, pw], bf16, tag="scs", name="scs")
                            nc.scalar.activation(
                                out=sc,
                                in_=t[:, pw * ph : pw * (ph + 1)],
                                func=AF.Copy,
                                accum_out=parts[:, pi + ph : pi + ph + 1],
                            )
                        for s in range(QPC):
                            Q = QPC * q + s
                            nc.tensor.matmul(
                                msum,
                                estrip[:, NQ - Q : 2 * NQ - Q],
                                t[:, QW * s : QW * (s + 1)],
                                start=(i_mm == 0),
                                stop=(i_mm == n_mm - 1),
                            )
                            i_mm += 1
                return msum, parts, xres

            def emit_chain(msum, parts, after):
                """pooled -> z -> kern -> banded 3x3 conv -> sigmoid ->
                staged s (partitions 0/64)."""
                pooled = small.tile([128, 2], f32, tag="pooled")
                r0 = nc.vector.reduce_sum(
                    out=pooled[:, 0:1], in_=parts[:, 0:NCH], axis=AX.X
                )
                r1 = nc.vector.reduce_sum(
                    out=pooled[:, 1:2],
                    in_=parts[:, NCH : 2 * NCH + 3],
                    axis=AX.X,
                )
                if after is not None:
                    # order-only deps: the whole chain of sample b hangs
                    # off these reduces, so this keeps the static scheduler
                    # from weaving chain_b (whose deps it mispredicts as
                    # early) into the multiply stream of sample b-1 on
                    # DVE/PE/ACT, which would head-of-line block stores
                    add_dep_helper(
                        r0.ins, after.ins, sync=False,
                        reason="chain_b after mults_{b-1}",
                    )
                    add_dep_helper(
                        r1.ins, after.ins, sync=False,
                        reason="chain_b after mults_{b-1}",
                    )
                z_sb = small.tile([128, 2], f32, tag="z")
                for j in range(2):
                    zp = ps.tile([128, 1], f32, tag="zsmall", name="zp")
                    for i in range(2):
                        nc.tensor.matmul(
                            zp,
                            w1_sb[:, i, 128 * j : 128 * (j + 1)],
                            pooled[:, i : i + 1],
                            start=(i == 0),
                            stop=(i == 1),
                        )
                    nc.scalar.activation(
                        out=z_sb[:, j : j + 1],
                        in_=zp,
                        func=AF.Relu,
                        bias=b1_sb[:, j : j + 1],
                        scale=1.0,
                    )
                kp = ps.tile([1, KS * KS], f32, tag="zsmall", name="kp")
                for j in range(2):
                    nc.tensor.matmul(
                        kp,
                        z_sb[:, j : j + 1],
                        w2_sb[:, j, :],
                        start=(j == 0),
                        stop=(j == 1),
                    )
                kern = small.tile([1, KS * KS], f32, tag="kern")
                nc.vector.tensor_add(out=kern, in0=kp, in1=b2_sb)
                kbp = ps.tile([128, KS * KS], f32, tag="zsmall", name="kbp")
                nc.tensor.matmul(kbp, ones_rf, kern, start=True, stop=True)
                kb = small.tile([128, KS * KS], f32, tag="kb")
                # fold the 1/C of the channel mean into the conv weights
                nc.scalar.activation(out=kb, in_=kbp, func=AF.Copy, scale=1.0 / c)

                # msum [NQ, 512] -> m_sq [h, w] on TensorE: 4 accumulating
                # selector matmuls scatter row q col 128r+w to partition
                # 4q+r (replaces a 128-descriptor DMA, 15.5us measured)
                m32 = small.tile([NQ, QW], bf16, tag="m32")
                nc.scalar.copy(out=m32, in_=msum)
                msq_p = pm.tile([h, w], f32, tag="msum", name="msqp")
                for r in range(RPB):
                    nc.tensor.matmul(
                        msq_p,
                        sels[r],
                        m32[:, w * r : w * (r + 1)],
                        start=(r == 0),
                        stop=(r == RPB - 1),
                    )
                m_sq = convt.tile([h, w], bf16, tag="msq")
                nc.scalar.copy(out=m_sq, in_=msq_p)

                # conv2d(m, kern) as 3 banded matmuls: for each kernel
                # column dx, T_dx[h,h'] = k[h-h'+1, dx] is tridiagonal;
                # att[:, w-shifted] += T_dx.T @ m[:, w-shifted].  Vertical
                # padding is implicit in the band clipping, horizontal
                # padding in the PSUM column offsets.
                tb = convt.tile([h, h], bf16, tag="tb")
                t_mats = []
                for dx in range(3):
                    T = convt.tile([h, h], bf16, tag=f"T{dx}", name="T")
                    nc.vector.tensor_scalar_mul(
                        out=T, in0=ident, scalar1=kb[:h, 3 + dx : 4 + dx]
                    )
                    nc.vector.tensor_scalar_mul(
                        out=tb, in0=d_up, scalar1=kb[:h, dx : dx + 1]
                    )
                    nc.vector.tensor_add(out=T, in0=T, in1=tb)
                    nc.vector.tensor_scalar_mul(
                        out=tb, in0=d_dn, scalar1=kb[:h, 6 + dx : 7 + dx]
                    )
                    nc.vector.tensor_add(out=T, in0=T, in1=tb)
                    t_mats.append(T)
                attp = pm.tile([h, w], f32, tag="msum", name="attp")
                nc.tensor.matmul(attp, t_mats[1], m_sq, start=True, stop=False)
                nc.tensor.matmul(
                    attp[:, 0 : w - 1],
                    t_mats[2],
                    m_sq[:, 1:w],
                    start=False,
                    stop=False,
                )
                nc.tensor.matmul(
                    attp[:, 1:w],
                    t_mats[0],
                    m_sq[:, 0 : w - 1],
                    start=False,
                    stop=True,
                )
                s_bf = convt.tile([h, w], bf16, tag="sbf")
                sig = nc.scalar.activation(out=s_bf, in_=attp, func=AF.Sigmoid)
                # stage s onto partitions 0/64 (legal matmul base
                # partitions) so the K=1 broadcast matmuls can read it;
                # two DMAs so the first half's broadcast matmuls don't
                # wait on the second half's staging
                sr = srp.tile([128, SRW], bf16, tag="sr", name="sr")
                nc.sync.dma_start(out=sr[0:1, :], in_=s_bf[0:64, :])
                nc.sync.dma_start(out=sr[64:65, :], in_=s_bf[64:128, :])
                return sr, sig

            def emit_mults(sr, xres, drain):
                """Broadcast s via K=1 matmuls, stage PSUM->SBUF bf16
                (split DVE/ACT), multiply x in place on VectorE (2x
                all-bf16)."""
                for m in range(NM):
                    bp = pb.tile([128, MW], f32, tag="bp", name="bp")
                    for s in range(MW // QW):
                        Q = (MW // QW) * m + s
                        r = 64 * (Q // QPR)
                        nc.tensor.matmul(
                            bp[:, QW * s : QW * (s + 1)],
                            ones_r[r : r + 1, :],
                            sr[r : r + 1, QW * (Q % QPR) : QW * (Q % QPR + 1)],
                            start=True,
                            stop=True,
                        )
                    bps = bsp.tile([128, MW], bf16, tag="bps", name="bps")
                    # staging split 12 DVE / 4 ACT: ACT carries all
                    # pooled partials, DVE the multiply stream.  In the
                    # drain (last sample) ACT has no future partials, so
                    # alternate 8/8 -- production rate directly bounds
                    # the final store stream there
                    on_act = (m % 2 == 1) if drain else (m % 4 == 3)
                    if on_act:
                        nc.scalar.copy(out=bps, in_=bp)
                    else:
                        nc.vector.tensor_copy(out=bps, in_=bp)
                    ct, co = (MW * m) // CH, (MW * m) % CH
                    for hh in range(2):
                        mi = nc.vector.tensor_mul(
                            out=xres[hh, ct][:, co : co + MW],
                            in0=xres[hh, ct][:, co : co + MW],
                            in1=bps,
                        )
                    if m == 5:
                        # anchor for the next chain's order-dep: far
                        # enough in that the scheduler can't hoist chain
                        # ops ahead of this multiply stream, early enough
                        # that the ~12us chain latency overlaps the tail
                        # of these multiplies instead of serializing
                        anchor = mi
                return anchor

            def emit_stores(b, xres, drain):
                """Stores (bf16, 1 MiB chunks) on the two HWDGE rings,
                overlapping the next samples' loads on the SWDGE queue.
                The drain sample also uses the then-idle gpsimd queue."""
                engs = (
                    [nc.scalar, nc.sync, nc.gpsimd]
                    if drain
                    else [nc.scalar, nc.sync]
                )
                i = 0
                for q in range(NS):
                    for hh in range(2):
                        engs[i % len(engs)].dma_start(
                            out=out_d[
                                b,
                                128 * hh : 128 * (hh + 1),
                                SW * q : SW * (q + 1),
                            ],
                            in_=xres[hh, q],
                        )
                        i += 1

            last_mult = None
            for b in range(bs):
                msum, parts, xres = emit_loads(b)
                # chain AND multiply ops are latency-critical (they gate
                # the store phase); high priority keeps the static
                # scheduler from burying them behind ready bulk work
                with tc.high_priority():
                    sr, _sig = emit_chain(msum, parts, last_mult)
                    last_mult = emit_mults(sr, xres, drain=(b == bs - 1))
                emit_stores(b, xres, drain=(b == bs - 1))

    nc.finalize()
    return nc


_NC_CACHE = {}


def _get_nc(key=(BS, C, H, W)):
    if key not in _NC_CACHE:
        _NC_CACHE[key] = build_nc(*key)
    return _NC_CACHE[key]


def _shard_inputs(x, w1, b1, w2, b2):
    import ml_dtypes

    xb = np.ascontiguousarray(x, dtype=np.float32).astype(ml_dtypes.bfloat16)
    in_maps = []
    for i in range(N_CORES):
        in_maps.append(
            {
                "x": np.ascontiguousarray(
                    xb[i * BS : (i + 1) * BS].reshape(BS, C, HW)
                ),
                "w1": np.ascontiguousarray(w1, dtype=np.float32),
                "b1": np.ascontiguousarray(b1, dtype=np.float32),
                "w2": np.ascontiguousarray(w2, dtype=np.float32),
                "b2": np.ascontiguousarray(b2, dtype=np.float32),
            }
        )
    return in_maps


def kernel(x, w1, b1, w2, b2):
    from concourse.bass_utils import run_bass_kernel_spmd

    nc = _get_nc()
    in_maps = _shard_inputs(x, w1, b1, w2, b2)
    res = run_bass_kernel_spmd(nc, in_maps, list(range(N_CORES)))
    out = np.concatenate(
        [
            np.asarray(r["out"]).astype(np.float32).reshape(BS, C, H, W)
            for r in res.results
        ],
        axis=0,
    )
    return out
